# revision 7
# baseline (speedup 1.0000x reference)
"""MoE (top-2 of 8 experts) Trainium2 kernel.

Strategy: expert-parallel across the 8 NeuronCores. The router (a tiny
[T,512]@[512,8] matmul + softmax + top-k, ~0.02% of the layer's FLOPs) runs
on host bit-identically to the reference (jax on CPU). Tokens are gathered
per expert on host, padded to a common capacity C, and each core computes
its expert's full FFN on device:

    outT = (w2.T @ gelu(w1.T @ xT + b1) + b2) * gate

in a transposed layout (features on partitions, tokens on the moving/free
axis) so both matmuls chain on the TensorEngine with no transposes, and the
b1/b2 biases are free per-partition operands. The gate multiply uses a
partition-broadcast gate row. Host scatter-adds the two expert
contributions per token back into the full [B,S,D] output.

Only the selected top-2 experts contribute to the reference output (the
gate is exactly zero elsewhere), so this computes 4x fewer FLOPs than the
dense reference while being numerically equivalent.
"""

import os
import sys

sys.path.insert(0, "/opt/trn_rl_repo")

import numpy as np

TOP_K = 2
N_CORES = 8
P = 128  # SBUF partitions

# Matmul dtype: "float32" (exact, 4 cyc/row) or "float32r" (1 cyc/row at
# N>=256, reduced internal precision). Overridable for experiments.
MM_DT = os.environ.get("MOE_MM_DT", "float32")
NTILE = 512  # moving-operand (token) tile; max for 4-byte dtypes
# n-tiles processed per weight pass (fp32/fp32r matmuls self-load weights,
# so >1 only helps dtypes with separate LDWEIGHTS)
NPAIR = int(os.environ.get("MOE_NPAIR", "1"))


def _route(x_flat, gate_w, gate_b):
    """Reference router, bit-identical: jax on CPU."""
    import jax
    import jax.numpy as jnp

    with jax.default_device(jax.devices("cpu")[0]):
        logits = jnp.asarray(x_flat) @ jnp.asarray(gate_w) + jnp.asarray(gate_b)
        raw_weights = jax.nn.softmax(logits, axis=-1)
        top_w, top_idx = jax.lax.top_k(raw_weights, TOP_K)
        return np.asarray(top_w), np.asarray(top_idx)


ACT_FUNC = os.environ.get("MOE_ACT_FUNC", "Gelu")  # CoreSim lacks Gelu; Tanh for sim


def _build_program(C, D, H, mm_dt_name):
    """Build the per-core Bass program (identical on all cores)."""
    import concourse.bass as bass
    import concourse.mybir as mybir
    import concourse.tile as tile
    from concourse import bacc

    f32 = mybir.dt.float32
    mm_dt = getattr(mybir.dt, mm_dt_name)
    KT = D // P  # 4  k-tiles for matmul1 (contraction over D)
    MT = H // P  # 16 m-tiles (H rows of hT)
    DT = D // P  # 4  d-tiles of the output
    NT = (C + NTILE - 1) // NTILE

    nc = bacc.Bacc(None, target_bir_lowering=False, debug=False)
    xt_h = nc.dram_tensor("xt", [D, C], f32, kind="ExternalInput")
    g_h = nc.dram_tensor("g", [1, C], f32, kind="ExternalInput")
    w1_h = nc.dram_tensor("w1", [D, H], f32, kind="ExternalInput")
    b1_h = nc.dram_tensor("b1", [P, MT], f32, kind="ExternalInput")
    w2_h = nc.dram_tensor("w2", [H, D], f32, kind="ExternalInput")
    b2_h = nc.dram_tensor("b2", [P, DT], f32, kind="ExternalInput")
    out_h = nc.dram_tensor("out", [D, C], f32, kind="ExternalOutput")

    w1_r = w1_h.ap().rearrange("(kt p) h -> p kt h", p=P)  # [128, KT, H]
    w2_r = w2_h.ap().rearrange("(mt p) d -> p mt d", p=P)  # [128, MT, D]
    xt_r = xt_h.ap().rearrange("(kt p) c -> p kt c", p=P)  # [128, KT, C]
    out_r = out_h.ap().rearrange("(dt p) c -> p dt c", p=P)  # [128, DT, C]

    with tile.TileContext(nc) as tc:
        with (
            tc.tile_pool(name="weights", bufs=1) as wpool,
            tc.tile_pool(name="xio", bufs=2) as xio,
            tc.tile_pool(name="gio", bufs=2) as gio,
            tc.tile_pool(name="oio", bufs=3) as oio,
            tc.tile_pool(name="hbuf", bufs=1) as hbuf,
            tc.tile_pool(name="ps1", bufs=2, space=bass.MemorySpace.PSUM) as ps1,
            tc.tile_pool(name="ps2", bufs=2, space=bass.MemorySpace.PSUM) as ps2,
        ):
            w1_sb = wpool.tile([P, KT, H], f32)
            for kt in range(KT):
                nc.sync.dma_start(out=w1_sb[:, kt, :], in_=w1_r[:, kt, :])
            w2_sb = wpool.tile([P, MT, D], f32)
            for mt in range(MT):
                nc.sync.dma_start(out=w2_sb[:, mt, :], in_=w2_r[:, mt, :])
            b1_sb = wpool.tile([P, MT], f32)
            nc.sync.dma_start(out=b1_sb, in_=b1_h.ap())
            b2_sb = wpool.tile([P, DT], f32)
            nc.sync.dma_start(out=b2_sb, in_=b2_h.ap())

            for n0 in range(0, NT, NPAIR):
                npair = min(NPAIR, NT - n0)
                # token slice covered by this group of n-tiles
                c0 = n0 * NTILE
                csz = min(NPAIR * NTILE, C - c0)
                xt_t = xio.tile([P, KT, csz], f32, tag="xt")
                nc.sync.dma_start(out=xt_t, in_=xt_r[:, :, c0 : c0 + csz])
                g_t = gio.tile([P, csz], f32, tag="g")
                nc.gpsimd.dma_start(
                    out=g_t, in_=g_h.ap()[:, c0 : c0 + csz].partition_broadcast(P)
                )
                hT = hbuf.tile([P, MT, csz], f32, tag="hT")
                nsz = [
                    min(NTILE, csz - i * NTILE)
                    for i in range((csz + NTILE - 1) // NTILE)
                ]
                for m in range(MT):
                    pst = [
                        ps1.tile([P, s], f32, tag=f"ps1_{i}", name=f"ps1_{i}")
                        for i, s in enumerate(nsz)
                    ]
                    for kt in range(KT):
                        lhs = w1_sb[:, kt, P * m : P * (m + 1)].bitcast(mm_dt)
                        for i, s in enumerate(nsz):
                            nc.tensor.matmul(
                                pst[i],
                                lhsT=lhs,
                                rhs=xt_t[:, kt, i * NTILE : i * NTILE + s].bitcast(
                                    mm_dt
                                ),
                                start=(kt == 0),
                                stop=(kt == KT - 1),
                            )
                    for i, s in enumerate(nsz):
                        nc.scalar.activation(
                            out=hT[:, m, i * NTILE : i * NTILE + s],
                            in_=pst[i],
                            func=getattr(mybir.ActivationFunctionType, ACT_FUNC),
                            bias=b1_sb[:, m : m + 1],
                            scale=1.0,
                        )
                for d in range(DT):
                    pso = [
                        ps2.tile([P, s], f32, tag=f"ps2_{i}", name=f"ps2_{i}")
                        for i, s in enumerate(nsz)
                    ]
                    for m in range(MT):
                        lhs = w2_sb[:, m, P * d : P * (d + 1)].bitcast(mm_dt)
                        for i, s in enumerate(nsz):
                            nc.tensor.matmul(
                                pso[i],
                                lhsT=lhs,
                                rhs=hT[:, m, i * NTILE : i * NTILE + s].bitcast(mm_dt),
                                start=(m == 0),
                                stop=(m == MT - 1),
                            )
                    ot = oio.tile([P, csz], f32, tag="ot")
                    for i, s in enumerate(nsz):
                        nc.vector.scalar_tensor_tensor(
                            out=ot[:, i * NTILE : i * NTILE + s],
                            in0=pso[i],
                            scalar=b2_sb[:, d : d + 1],
                            in1=g_t[:, i * NTILE : i * NTILE + s],
                            op0=mybir.AluOpType.add,
                            op1=mybir.AluOpType.mult,
                        )
                    nc.sync.dma_start(out=out_r[:, d, c0 : c0 + csz], in_=ot)

    nc.compile()
    return nc


def _run(nc, in_maps, trace=False):
    from concourse.bass_utils import run_bass_kernel_spmd

    if trace:
        # register the NTFF profiling hook (missing antenv.axon_hooks shim)
        import types

        import antenv

        if not hasattr(antenv, "axon_hooks"):
            mod = types.ModuleType("antenv.axon_hooks")
            _hook = [None]
            mod.set_axon_ntff_profile_hook = lambda h: _hook.__setitem__(0, h)
            mod.get_axon_ntff_profile_hook = lambda: _hook[0]
            sys.modules["antenv.axon_hooks"] = mod
            antenv.axon_hooks = mod
            from trn_agent_boot.trn_boot import _ntff_profile_via_ctypes

            mod.set_axon_ntff_profile_hook(
                _ntff_profile_via_ctypes("/opt/axon/libaxon_pjrt.so")
            )
    return run_bass_kernel_spmd(
        nc, in_maps, core_ids=list(range(N_CORES)), trace=trace
    )


def kernel(x, gate_w, gate_b, w1, b1, w2, b2, _trace=False):
    x = np.ascontiguousarray(np.asarray(x, dtype=np.float32))
    gate_w = np.asarray(gate_w, dtype=np.float32)
    gate_b = np.asarray(gate_b, dtype=np.float32)
    w1 = np.asarray(w1, dtype=np.float32)
    b1 = np.asarray(b1, dtype=np.float32)
    w2 = np.asarray(w2, dtype=np.float32)
    b2 = np.asarray(b2, dtype=np.float32)

    B, S, D = x.shape
    E = gate_w.shape[1]
    H = w1.shape[2]
    assert E == N_CORES
    T = B * S
    x_flat = x.reshape(T, D)

    top_w, top_idx = _route(x_flat, gate_w, gate_b)

    toks, gvals = [], []
    for e in range(E):
        mask = top_idx == e  # [T, K]; at most one True per row
        t_ids = np.nonzero(mask.any(axis=1))[0]
        toks.append(t_ids)
        gvals.append(top_w[mask].astype(np.float32))
    Cmax = max(len(t) for t in toks)
    C = max(((Cmax + P - 1) // P) * P, NTILE)

    in_maps = []
    for e in range(E):
        cnt = len(toks[e])
        XT = np.zeros((D, C), np.float32)
        XT[:, :cnt] = x_flat[toks[e]].T
        G = np.zeros((1, C), np.float32)
        G[0, :cnt] = gvals[e]
        MT, DT = H // P, D // P
        in_maps.append(
            {
                "xt": XT,
                "g": G,
                "w1": np.ascontiguousarray(w1[e]),
                "b1": np.ascontiguousarray(b1[e].reshape(MT, P).T),
                "w2": np.ascontiguousarray(w2[e]),
                "b2": np.ascontiguousarray(b2[e].reshape(DT, P).T),
            }
        )

    nc = _build_program(C, D, H, MM_DT)
    res = _run(nc, in_maps, trace=_trace)

    out_flat = np.zeros((T, D), np.float32)
    for e in range(E):
        cnt = len(toks[e])
        out_flat[toks[e]] += res.results[e]["out"][:, :cnt].T

    out = out_flat.reshape(B, S, D)
    if _trace:
        return out, res.exec_time_ns
    return out


# revision 9
# speedup vs baseline: 2.8023x; 2.8023x over previous
"""MoE (top-2 of 8 experts) Trainium2 kernel.

Strategy: expert-parallel across the 8 NeuronCores. The router (a tiny
[T,512]@[512,8] matmul + softmax + top-k, ~0.02% of the layer's FLOPs) runs
on host bit-identically to the reference (jax on CPU). Tokens are gathered
per expert on host, padded to a common capacity C, and each core computes
its expert's full FFN on device:

    outT = (w2.T @ gelu(w1.T @ xT + b1) + b2) * gate

in a transposed layout (features on partitions, tokens on the moving/free
axis) so both matmuls chain on the TensorEngine with no transposes, and the
b1/b2 biases are free per-partition operands. The gate multiply uses a
partition-broadcast gate row. Host scatter-adds the two expert
contributions per token back into the full [B,S,D] output.

Only the selected top-2 experts contribute to the reference output (the
gate is exactly zero elsewhere), so this computes 4x fewer FLOPs than the
dense reference while being numerically equivalent.
"""

import os
import sys

sys.path.insert(0, "/opt/trn_rl_repo")

import numpy as np

TOP_K = 2
N_CORES = 8
P = 128  # SBUF partitions

# Matmul dtype: "float32" (exact, 4 cyc/row) or "float32r" (1 cyc/row at
# N>=256, reduced internal precision). Overridable for experiments.
MM_DT = os.environ.get("MOE_MM_DT", "float32")
NTILE = 512  # moving-operand (token) tile; max for 4-byte dtypes
# n-tiles processed per weight pass (fp32/fp32r matmuls self-load weights,
# so >1 only helps dtypes with separate LDWEIGHTS)
NPAIR = int(os.environ.get("MOE_NPAIR", "1"))


def _route(x_flat, gate_w, gate_b):
    """Reference router, bit-identical: jax on CPU."""
    import jax
    import jax.numpy as jnp

    with jax.default_device(jax.devices("cpu")[0]):
        logits = jnp.asarray(x_flat) @ jnp.asarray(gate_w) + jnp.asarray(gate_b)
        raw_weights = jax.nn.softmax(logits, axis=-1)
        top_w, top_idx = jax.lax.top_k(raw_weights, TOP_K)
        return np.asarray(top_w), np.asarray(top_idx)


ACT_FUNC = os.environ.get("MOE_ACT_FUNC", "Gelu")  # CoreSim lacks Gelu; Tanh for sim


def _build_program(C, D, H, mm_dt_name):
    """Build the per-core Bass program (identical on all cores)."""
    import concourse.bass as bass
    import concourse.mybir as mybir
    import concourse.tile as tile
    from concourse import bacc

    f32 = mybir.dt.float32
    mm_dt = getattr(mybir.dt, mm_dt_name)
    KT = D // P  # 4  k-tiles for matmul1 (contraction over D)
    MT = H // P  # 16 m-tiles (H rows of hT)
    DT = D // P  # 4  d-tiles of the output
    NT = (C + NTILE - 1) // NTILE

    nc = bacc.Bacc(None, target_bir_lowering=False, debug=False)
    xt_h = nc.dram_tensor("xt", [D, C], mm_dt, kind="ExternalInput")
    g_h = nc.dram_tensor("g", [1, C], f32, kind="ExternalInput")
    w1_h = nc.dram_tensor("w1", [D, H], mm_dt, kind="ExternalInput")
    b1_h = nc.dram_tensor("b1", [P, MT], f32, kind="ExternalInput")
    w2_h = nc.dram_tensor("w2", [H, D], mm_dt, kind="ExternalInput")
    b2_h = nc.dram_tensor("b2", [P, DT], f32, kind="ExternalInput")
    out_h = nc.dram_tensor("out", [D, C], f32, kind="ExternalOutput")

    w1_r = w1_h.ap().rearrange("(kt p) h -> p kt h", p=P)  # [128, KT, H]
    w2_r = w2_h.ap().rearrange("(mt p) d -> p mt d", p=P)  # [128, MT, D]
    xt_r = xt_h.ap().rearrange("(kt p) c -> p kt c", p=P)  # [128, KT, C]
    out_r = out_h.ap().rearrange("(dt p) c -> p dt c", p=P)  # [128, DT, C]

    with tile.TileContext(nc) as tc:
        with (
            tc.tile_pool(name="weights", bufs=1) as wpool,
            tc.tile_pool(name="xio", bufs=2) as xio,
            tc.tile_pool(name="gio", bufs=2) as gio,
            tc.tile_pool(name="oio", bufs=3) as oio,
            tc.tile_pool(name="hbuf", bufs=1) as hbuf,
            tc.tile_pool(name="ps1", bufs=2, space=bass.MemorySpace.PSUM) as ps1,
            tc.tile_pool(name="ps2", bufs=2, space=bass.MemorySpace.PSUM) as ps2,
        ):
            w1_sb = wpool.tile([P, KT, H], mm_dt)
            for kt in range(KT):
                nc.sync.dma_start(out=w1_sb[:, kt, :], in_=w1_r[:, kt, :])
            w2_sb = wpool.tile([P, MT, D], mm_dt)
            for mt in range(MT):
                nc.sync.dma_start(out=w2_sb[:, mt, :], in_=w2_r[:, mt, :])
            b1_sb = wpool.tile([P, MT], f32)
            nc.sync.dma_start(out=b1_sb, in_=b1_h.ap())
            b2_sb = wpool.tile([P, DT], f32)
            nc.sync.dma_start(out=b2_sb, in_=b2_h.ap())

            for n0 in range(0, NT, NPAIR):
                npair = min(NPAIR, NT - n0)
                # token slice covered by this group of n-tiles
                c0 = n0 * NTILE
                csz = min(NPAIR * NTILE, C - c0)
                xt_t = xio.tile([P, KT, csz], mm_dt, tag="xt")
                nc.sync.dma_start(out=xt_t, in_=xt_r[:, :, c0 : c0 + csz])
                g_t = gio.tile([P, csz], f32, tag="g")
                nc.gpsimd.dma_start(
                    out=g_t, in_=g_h.ap()[:, c0 : c0 + csz].partition_broadcast(P)
                )
                hT = hbuf.tile([P, MT, csz], mm_dt, tag="hT")
                nsz = [
                    min(NTILE, csz - i * NTILE)
                    for i in range((csz + NTILE - 1) // NTILE)
                ]
                for m in range(MT):
                    pst = [
                        ps1.tile([P, s], f32, tag=f"ps1_{i}", name=f"ps1_{i}")
                        for i, s in enumerate(nsz)
                    ]
                    for kt in range(KT):
                        lhs = w1_sb[:, kt, P * m : P * (m + 1)]
                        for i, s in enumerate(nsz):
                            nc.tensor.matmul(
                                pst[i],
                                lhsT=lhs,
                                rhs=xt_t[:, kt, i * NTILE : i * NTILE + s],
                                start=(kt == 0),
                                stop=(kt == KT - 1),
                            )
                    for i, s in enumerate(nsz):
                        nc.scalar.activation(
                            out=hT[:, m, i * NTILE : i * NTILE + s],
                            in_=pst[i],
                            func=getattr(mybir.ActivationFunctionType, ACT_FUNC),
                            bias=b1_sb[:, m : m + 1],
                            scale=1.0,
                        )
                for d in range(DT):
                    pso = [
                        ps2.tile([P, s], f32, tag=f"ps2_{i}", name=f"ps2_{i}")
                        for i, s in enumerate(nsz)
                    ]
                    for m in range(MT):
                        lhs = w2_sb[:, m, P * d : P * (d + 1)]
                        for i, s in enumerate(nsz):
                            nc.tensor.matmul(
                                pso[i],
                                lhsT=lhs,
                                rhs=hT[:, m, i * NTILE : i * NTILE + s],
                                start=(m == 0),
                                stop=(m == MT - 1),
                            )
                    ot = oio.tile([P, csz], f32, tag="ot")
                    for i, s in enumerate(nsz):
                        nc.vector.scalar_tensor_tensor(
                            out=ot[:, i * NTILE : i * NTILE + s],
                            in0=pso[i],
                            scalar=b2_sb[:, d : d + 1],
                            in1=g_t[:, i * NTILE : i * NTILE + s],
                            op0=mybir.AluOpType.add,
                            op1=mybir.AluOpType.mult,
                        )
                    nc.sync.dma_start(out=out_r[:, d, c0 : c0 + csz], in_=ot)

    nc.compile()
    return nc


def _run(nc, in_maps, trace=False):
    from concourse.bass_utils import run_bass_kernel_spmd

    if trace:
        # register the NTFF profiling hook (missing antenv.axon_hooks shim)
        import types

        import antenv

        if not hasattr(antenv, "axon_hooks"):
            mod = types.ModuleType("antenv.axon_hooks")
            _hook = [None]
            mod.set_axon_ntff_profile_hook = lambda h: _hook.__setitem__(0, h)
            mod.get_axon_ntff_profile_hook = lambda: _hook[0]
            sys.modules["antenv.axon_hooks"] = mod
            antenv.axon_hooks = mod
            from trn_agent_boot.trn_boot import _ntff_profile_via_ctypes

            mod.set_axon_ntff_profile_hook(
                _ntff_profile_via_ctypes("/opt/axon/libaxon_pjrt.so")
            )
    return run_bass_kernel_spmd(
        nc, in_maps, core_ids=list(range(N_CORES)), trace=trace
    )


def kernel(x, gate_w, gate_b, w1, b1, w2, b2, _trace=False):
    x = np.ascontiguousarray(np.asarray(x, dtype=np.float32))
    gate_w = np.asarray(gate_w, dtype=np.float32)
    gate_b = np.asarray(gate_b, dtype=np.float32)
    w1 = np.asarray(w1, dtype=np.float32)
    b1 = np.asarray(b1, dtype=np.float32)
    w2 = np.asarray(w2, dtype=np.float32)
    b2 = np.asarray(b2, dtype=np.float32)

    B, S, D = x.shape
    E = gate_w.shape[1]
    H = w1.shape[2]
    assert E == N_CORES
    T = B * S
    x_flat = x.reshape(T, D)

    top_w, top_idx = _route(x_flat, gate_w, gate_b)

    toks, gvals = [], []
    for e in range(E):
        mask = top_idx == e  # [T, K]; at most one True per row
        t_ids = np.nonzero(mask.any(axis=1))[0]
        toks.append(t_ids)
        gvals.append(top_w[mask].astype(np.float32))
    Cmax = max(len(t) for t in toks)
    C = max(((Cmax + P - 1) // P) * P, NTILE)

    in_maps = []
    for e in range(E):
        cnt = len(toks[e])
        XT = np.zeros((D, C), np.float32)
        XT[:, :cnt] = x_flat[toks[e]].T
        G = np.zeros((1, C), np.float32)
        G[0, :cnt] = gvals[e]
        MT, DT = H // P, D // P
        in_maps.append(
            {
                "xt": XT,
                "g": G,
                "w1": np.ascontiguousarray(w1[e]),
                "b1": np.ascontiguousarray(b1[e].reshape(MT, P).T),
                "w2": np.ascontiguousarray(w2[e]),
                "b2": np.ascontiguousarray(b2[e].reshape(DT, P).T),
            }
        )

    nc = _build_program(C, D, H, MM_DT)
    res = _run(nc, in_maps, trace=_trace)

    out_flat = np.zeros((T, D), np.float32)
    for e in range(E):
        cnt = len(toks[e])
        out_flat[toks[e]] += res.results[e]["out"][:, :cnt].T

    out = out_flat.reshape(B, S, D)
    if _trace:
        return out, res.exec_time_ns
    return out
